# revision 12
# baseline (speedup 1.0000x reference)
"""GCN encoder (2x GCNConv + PReLU, averaged) on 8 Trainium2 NeuronCores.

Math (per conv):
    deg[c]  = sum_{e: col_e = c} ew_e + 1
    dinv    = 1/sqrt(deg)
    hhat    = dinv * (x @ W)                       (row-scaled)
    agg[c]  = sum_{e: col_e = c} ew_e * hhat[row_e] + hhat[c]
    out     = prelu(dinv * agg + b, a)
final = (out1 + out2) / 2

v2 vs baseline: the whole aggregation pipeline runs in bf16 (hhat storage,
gathers, and both matmul operands — fp32 matmul costs 4 cycles/row vs 1 for
bf16, and bf16 halves all streaming DMA bytes).  The per-tile selection
matrices S[e, m] = ew_e * (lcol_e == m) are no longer precomputed on the host
and streamed from DRAM (64KB per 128-edge tile); instead each tile's S is
built on-chip by a single fused DVE op
    S = (iota == lcol) * ew        (tensor_scalar, 2 per-partition AP scalars)
from 8 bytes/edge of compact lcol/ew data.  deg -> dinv is computed on the
host (np.bincount) and uploaded (~200KB/graph), dropping the padded degree
tables and the on-device reduction entirely.

Device strategy (unchanged): target nodes sharded over 8 cores (6272 nodes
each on a 128-padded grid, N_PAD = 50176 = 8*49*128). Edges are routed on the
host to the core owning their target col, sorted by col, and laid out as
padded 128-edge tiles grouped into 128-col windows. Self-loops become plain
edges (ew=1, row=col) thanks to the dinv factorization. Source rows are
gathered with the SWDGE dma_gather instruction (int16 indices), so each
conv's edges are split into two streams by source-row half (< / >= N_PAD/2);
both streams of a window accumulate into the same PSUM bank.
"""
import os
import sys

# run_bass_kernel_spmd executes through the axon PJRT platform; if the
# caller pinned jax to cpu, lift that before jax gets imported below.
_jp = os.environ.get("JAX_PLATFORMS")
if _jp is not None and "axon" not in _jp and "neuron" not in _jp:
    del os.environ["JAX_PLATFORMS"]

sys.path.insert(0, "/opt/trn_rl_repo/concourse")
sys.path.insert(0, "/opt/trn_rl_repo")

import ml_dtypes
import numpy as np

import concourse.bass as bass
import concourse.bacc as bacc
import concourse.mybir as mybir
import concourse.tile as tile
from concourse.tile_rust import add_dep_helper
from concourse.bass_utils import run_bass_kernel_spmd

P = 128
N = 50000
NCORES = 8
SHARD_TILES = 49                      # node tiles per core
NT = NCORES * SHARD_TILES             # 392 node tiles
N_PAD = NT * P                        # 50176
HALF = N_PAD // 2                     # 25088 (< 2**15 for int16 gather idx)
SHARD = SHARD_TILES * P               # 6272
C1, C2, H = 512, 256, 256
GD = 16                               # gather blocks (tiles) per dma_gather
NI = P * GD                           # rows per dma_gather
NQ = 4                                # SWDGE queues
CHUNK = 4                             # super-tiles per stream DMA
SPAN = 1024                           # nodes per xT load (2KB/partition rows)
F32 = mybir.dt.float32
BF16 = mybir.dt.bfloat16
I16 = mybir.dt.int16
BF = ml_dtypes.bfloat16

LAST_EXEC_NS = None                   # set when BASS_KERNEL_TRACE=1


def _stream_layout(rows_l, cols, wts, k_arr, w_arr):
    """Slot/pad layout for one (graph, half) edge stream, all cores.

    Returns eidx16 [NCORES,128,NS*NI/16] (wrapped+replicated), lc/ew
    [NCORES,128,NS*GD] f32 (per-slot local col / edge weight, one column
    per 128-edge tile), tpw [SHARD_TILES], NS.
    """
    kw = k_arr * SHARD_TILES + w_arr
    order = np.argsort(kw, kind="stable")
    rows_l, wts, kw = rows_l[order], wts[order], kw[order]
    lcol = (cols[order] & 127).astype(np.float32)

    cw = np.bincount(kw, minlength=NCORES * SHARD_TILES).reshape(
        NCORES, SHARD_TILES
    )
    tpw = np.maximum((cw.max(axis=0) + P - 1) // P, 1).astype(np.int64)
    woff = np.concatenate([[0], np.cumsum(tpw * P)])
    L = int(woff[-1])
    NS = (L // P + GD - 1) // GD
    L_pad = NS * GD * P

    kw_start = np.searchsorted(kw, np.arange(NCORES * SHARD_TILES))
    rank_kw = np.arange(len(kw)) - kw_start[kw]
    slot = woff[kw % SHARD_TILES] + rank_kw

    eidx = np.zeros((NCORES, L_pad), np.int16)
    lc = np.zeros((NCORES, L_pad), np.float32)
    ew = np.zeros((NCORES, L_pad), np.float32)
    k_of = kw // SHARD_TILES
    for k in range(NCORES):
        sel = k_of == k
        sl = slot[sel]
        eidx[k, sl] = rows_l[sel].astype(np.int16)
        lc[k, sl] = lcol[sel]
        ew[k, sl] = wts[sel]

    # per-slot arrays -> [k, p, tile]: slot j = tile*(j//128) lane (j%128)
    ntiles = L_pad // P
    lc_d = np.ascontiguousarray(
        lc.reshape(NCORES, ntiles, P).transpose(0, 2, 1))
    ew_d = np.ascontiguousarray(
        ew.reshape(NCORES, ntiles, P).transpose(0, 2, 1))

    # idx: slot j in super s at (j%16, j//16), replicated over 8 groups
    w16 = eidx.reshape(NCORES, NS, NI // 16, 16).transpose(0, 1, 3, 2)
    w16 = np.tile(w16, (1, 1, 8, 1))                     # [k, NS, 128, NI/16]
    eidx_d = np.ascontiguousarray(w16.transpose(0, 2, 1, 3)).reshape(
        NCORES, P, NS * (NI // 16)
    )
    return eidx_d, lc_d, ew_d, tpw.tolist(), NS


def _prep_graph(x, ei, ew, W, C):
    """Host marshaling for one graph. Returns dict of device arrays + meta."""
    rows0 = np.asarray(ei[0], dtype=np.int64)
    cols0 = np.asarray(ei[1], dtype=np.int64)
    w0 = np.asarray(ew, dtype=np.float32)
    # self loops as plain edges (ew=1, row=col), for every padded node
    loop = np.arange(N_PAD, dtype=np.int64)
    rows = np.concatenate([rows0, loop])
    cols = np.concatenate([cols0, loop])
    wts = np.concatenate([w0, np.ones(N_PAD, np.float32)])

    # deg -> dinv on host (self-loop weights already included)
    deg = np.bincount(cols, weights=wts.astype(np.float64), minlength=N_PAD)
    dinv = (1.0 / np.sqrt(deg)).astype(np.float32)      # deg >= 1 always
    dinv_t = np.ascontiguousarray(dinv.reshape(NT, P).T)  # [P, NT]

    # fold dinv[row] into the edge weight so hhat = x @ W needs no
    # per-node scale on device (self-loop weight 1 becomes dinv[col])
    wts = wts * dinv[rows]

    order = np.argsort(cols, kind="stable")
    rows, cols, wts = rows[order], cols[order], wts[order]

    gt = cols >> 7
    k_arr = gt // SHARD_TILES
    w_arr = gt % SHARD_TILES

    streams = []
    for half in (0, 1):
        sel = (rows >= HALF) == bool(half)
        streams.append(_stream_layout(
            rows[sel] - half * HALF, cols[sel], wts[sel],
            k_arr[sel], w_arr[sel],
        ))

    xT = np.zeros((C, N_PAD), BF)
    xT[:, :N] = np.asarray(x, np.float32).T.astype(BF)
    Wd = np.ascontiguousarray(
        np.asarray(W, np.float32).astype(BF).reshape(
            C // P, P, H).transpose(1, 0, 2)
    ).reshape(P, (C // P) * H)

    meta = {
        "tpw": [streams[0][3], streams[1][3]],
        "NS": [streams[0][4], streams[1][4]],
    }
    arrs = {
        "dinv": dinv_t, "dinv_flat": dinv,
        "eidx": [streams[0][0], streams[1][0]],
        "lc": [streams[0][1], streams[1][1]],
        "ew": [streams[0][2], streams[1][2]],
        "xT": xT, "W": Wd,
    }
    return arrs, meta


def _build(meta1, meta2, b1_nonzero, b2_nonzero, a1_uniform, a2_uniform):
    nc = bacc.Bacc("TRN2", target_bir_lowering=False, debug=False,
                   num_devices=NCORES, num_swdge_queues=NQ)
    Pool = mybir.EngineType.Pool

    # ---- DRAM parameters ----
    xT1 = nc.declare_dram_parameter("xT1", [C1, SHARD], BF16, isOutput=False)
    xT2 = nc.declare_dram_parameter("xT2", [C2, SHARD], BF16, isOutput=False)
    W1 = nc.declare_dram_parameter("W1", [P, (C1 // P) * H], BF16, isOutput=False)
    W2 = nc.declare_dram_parameter("W2", [P, (C2 // P) * H], BF16, isOutput=False)
    dh12 = nc.declare_dram_parameter("dh12", [P, 4 * SHARD_TILES], F32,
                                     isOutput=False)
    eidx, lcp, ewp = [], [], []
    for ci, meta in ((0, meta1), (1, meta2)):
        for st in (0, 1):
            NS = meta["NS"][st]
            eidx.append(nc.declare_dram_parameter(
                f"eidx{ci}{st}", [P, NS * (NI // 16)], I16, isOutput=False))
            lcp.append(nc.declare_dram_parameter(
                f"lc{ci}{st}", [P, NS * GD], F32, isOutput=False))
            ewp.append(nc.declare_dram_parameter(
                f"ew{ci}{st}", [P, NS * GD], F32, isOutput=False))
    iota_in = nc.declare_dram_parameter("iota", [P, P], BF16, isOutput=False)
    bvec = nc.declare_dram_parameter("bvec", [P, 2 * H], F32, isOutput=False)
    avec = nc.declare_dram_parameter("avec", [P, 2 * H], F32, isOutput=False)
    out = nc.declare_dram_parameter("out", [SHARD, H], F32, isOutput=True)

    hh1o = nc.dram_tensor("hh1o", [SHARD, H], BF16)
    hh2o = nc.dram_tensor("hh2o", [SHARD, H], BF16)
    hh1 = nc.dram_tensor("hh1", [N_PAD, H], BF16)
    hh2 = nc.dram_tensor("hh2", [N_PAD, H], BF16)

    qrr = [0]  # round-robin SWDGE queue counter

    with tile.TileContext(nc) as tc:
        with (
            tc.tile_pool(name="const", bufs=1) as cpool,
            tc.tile_pool(name="dinv", bufs=1) as dvpool,
            tc.tile_pool(name="xin", bufs=2) as xpool,
            tc.tile_pool(name="hout", bufs=2) as hpool,
            tc.tile_pool(name="stream", bufs=2) as spool,
            tc.tile_pool(name="gath", bufs=3) as gpool,
            tc.tile_pool(name="sld", bufs=2) as sldpool,
            tc.tile_pool(name="evict", bufs=2) as epool,
            tc.tile_pool(name="hps", bufs=2, space="PSUM") as hps,
            tc.tile_pool(name="wps", bufs=2, space="PSUM") as wps,
        ):
            iota_t = cpool.tile([P, P], BF16)
            nc.sync.dma_start(out=iota_t[:], in_=iota_in[:])
            w1_t = cpool.tile([P, (C1 // P) * H], BF16)
            nc.sync.dma_start(out=w1_t[:], in_=W1[:])
            w2_t = cpool.tile([P, (C2 // P) * H], BF16)
            nc.sync.dma_start(out=w2_t[:], in_=W2[:])
            b_t = cpool.tile([P, 2 * H], F32)
            if b1_nonzero or b2_nonzero:
                nc.sync.dma_start(out=b_t[:], in_=bvec[:])
            a_t = cpool.tile([P, 2 * H], F32)
            if a1_uniform is None or a2_uniform is None:
                nc.sync.dma_start(out=a_t[:], in_=avec[:])

            # ---- dinv-derived evict scales (host-computed) ----
            dh_t = dvpool.tile([P, 4 * SHARD_TILES], F32)
            nc.sync.dma_start(out=dh_t[:], in_=dh12[:])
            ST = SHARD_TILES
            dh1 = dh_t[:, 0:ST]
            dh2 = dh_t[:, ST:2 * ST]
            dha1 = dh_t[:, 2 * ST:3 * ST]
            dha2 = dh_t[:, 3 * ST:4 * ST]
            dhan_t = dvpool.tile([P, 2 * ST], F32)
            nc.vector.tensor_scalar_mul(
                out=dhan_t[:], in0=dh_t[:, 2 * ST:4 * ST], scalar1=-1.0)
            dhan1 = dhan_t[:, 0:ST]
            dhan2 = dhan_t[:, ST:2 * ST]

            # ---- hhat = x @ W (dinv folded into edge weights) ----
            PSPAN = 4                 # node tiles per PSUM tile (2 banks)

            def h_phase(xT, w_t, nck, hh, lo, hi):
                writes = []
                for n0 in range(lo, hi, SPAN):
                    n1 = min(n0 + SPAN, hi)
                    xt = xpool.tile([P, nck * (n1 - n0)], BF16, tag="xt")
                    nc.sync.dma_start(
                        out=xt[:].rearrange("p (c n) -> p c n", c=nck),
                        in_=xT[:, n0:n1].rearrange("(c p) n -> p c n", p=P),
                    )
                    njt = (n1 - n0) // P
                    for j0 in range(0, njt, PSPAN):
                        jn = min(PSPAN, njt - j0)
                        ps = hps.tile([P, PSPAN * H], F32, tag="hps")
                        for j in range(j0, j0 + jn):
                            for c in range(nck):
                                nc.tensor.matmul(
                                    out=ps[:, (j - j0) * H:(j - j0 + 1) * H],
                                    lhsT=xt[:, c * (n1 - n0) + j * P:
                                            c * (n1 - n0) + (j + 1) * P],
                                    rhs=w_t[:, c * H:(c + 1) * H],
                                    start=(c == 0), stop=(c == nck - 1),
                                )
                        ht = hpool.tile([P, PSPAN * H], BF16, tag="ht")
                        nc.scalar.activation(
                            out=ht[:, :jn * H], in_=ps[:, :jn * H],
                            func=mybir.ActivationFunctionType.Copy,
                        )
                        t0 = (n0 // P) + j0
                        wr = nc.sync.dma_start(
                            out=hh[t0 * P:(t0 + jn) * P, :].rearrange(
                                "(t p) h -> p t h", p=P),
                            in_=ht[:, :jn * H].rearrange(
                                "p (t h) -> p t h", h=H),
                        )
                        writes.append(wr)
                return writes

            # compute only this core's node shard, then AllGather the full
            # hhat across the 8 cores (replicated result in each local hh)
            wr1 = h_phase(xT1, w1_t, C1 // P, hh1o, 0, SHARD)
            wr2 = h_phase(xT2, w2_t, C2 // P, hh2o, 0, SHARD)

            barriers = {}
            rg = [list(range(NCORES))]
            for key, wrs, own, full in [("1", wr1, hh1o, hh1),
                                        ("2", wr2, hh2o, hh2)]:
                cc = nc.gpsimd.collective_compute(
                    kind="AllGather",
                    op=mybir.AluOpType.bypass,
                    replica_groups=rg,
                    ins=[own[:, :]],
                    outs=[full[:, :]],
                )
                for w in wrs:
                    add_dep_helper(cc.ins, w.ins, reason=f"hh{key} own write")
                bar = nc.engines[Pool].nop(nofuse=True, hint=f"hh{key}_ready")
                add_dep_helper(bar.ins, cc.ins, reason=f"hh{key} allgather")
                barriers[key] = bar

            # ---- edge streams ----
            class Stream:
                def __init__(self, eidx, lcp, ewp, hh_half, NS, barrier, tag):
                    self.eidx, self.lcp, self.ewp = eidx, lcp, ewp
                    self.hh = hh_half
                    self.NS, self.barrier, self.tag = NS, barrier, tag
                    self.t_mm = 0
                    self.chunk_base = 0
                    self.idx_tile = None
                    self.lc_tile = None
                    self.ew_tile = None
                    self.s_tile = None
                    self.g_tile = None

                def ensure(self):
                    s, g = divmod(self.t_mm, GD)
                    if s % CHUNK == 0 and g == 0:
                        s1 = min(s + CHUNK, self.NS)
                        it = spool.tile([P, (s1 - s) * (NI // 16)], I16,
                                        tag=f"idx{self.tag}")
                        nc.sync.dma_start(
                            out=it[:],
                            in_=self.eidx[:, s * (NI // 16):s1 * (NI // 16)])
                        self.idx_tile = it
                        lt = spool.tile([P, (s1 - s) * GD], F32,
                                        tag=f"lc{self.tag}")
                        nc.sync.dma_start(
                            out=lt[:], in_=self.lcp[:, s * GD:s1 * GD])
                        self.lc_tile = lt
                        et = spool.tile([P, (s1 - s) * GD], F32,
                                        tag=f"ew{self.tag}")
                        nc.sync.dma_start(
                            out=et[:], in_=self.ewp[:, s * GD:s1 * GD])
                        self.ew_tile = et
                        self.chunk_base = s
                    if g == 0:
                        so = s - self.chunk_base
                        gt_ = gpool.tile([P, GD * H], BF16, tag=f"g{self.tag}")
                        gi = nc.gpsimd.dma_gather(
                            out_ap=gt_[:].rearrange("p (b e) -> p b e", e=H),
                            in_ap=self.hh,
                            idxs_ap=self.idx_tile[
                                :, so * (NI // 16):(so + 1) * (NI // 16)],
                            num_idxs=NI,
                            num_idxs_reg=NI,
                            elem_size=H,
                            queue_num=qrr[0] % NQ,
                            single_packet=False,
                        )
                        qrr[0] += 1
                        add_dep_helper(
                            gi.ins, self.barrier.ins, reason="hh ready")
                        self.g_tile = gt_
                        st_ = sldpool.tile([P, GD * P], BF16,
                                           tag=f"sl{self.tag}")
                        self.s_tile = st_

                def prep_tile(self):
                    self.ensure()
                    s, g = divmod(self.t_mm, GD)
                    col = (s - self.chunk_base) * GD + g
                    # S[e, m] = ew_e * (lcol_e == m), built on-chip
                    nc.vector.tensor_scalar(
                        out=self.s_tile[:, g * P:(g + 1) * P],
                        in0=iota_t[:],
                        scalar1=self.lc_tile[:, col:col + 1],
                        scalar2=self.ew_tile[:, col:col + 1],
                        op0=mybir.AluOpType.is_equal,
                        op1=mybir.AluOpType.mult,
                    )
                    self.t_mm += 1
                    return (self.s_tile, self.g_tile, g)

            streams = []
            for ci, (meta, hh) in enumerate([(meta1, hh1), (meta2, hh2)]):
                pair = []
                for st in (0, 1):
                    half_ap = hh[st * HALF:(st + 1) * HALF, :]
                    bar = barriers[f"{ci + 1}"]
                    pair.append(Stream(
                        eidx[ci * 2 + st], lcp[ci * 2 + st], ewp[ci * 2 + st],
                        half_ap, meta["NS"][st], bar, f"{ci}{st}"))
                streams.append(pair)

            # per-stream tiles per window, with drain pads in the last window
            tpws = []
            for ci, meta in ((0, meta1), (1, meta2)):
                pair = []
                for st in (0, 1):
                    tp = list(meta["tpw"][st])
                    tp[-1] += meta["NS"][st] * GD - sum(tp)
                    pair.append(tp)
                tpws.append(pair)

            def evict(ps, dh, dha, dhan, wdx, aun, b_nz, boff):
                q = epool.tile([P, H], F32, tag="q")
                r = epool.tile([P, H], F32, tag="r")
                if not b_nz and aun is not None:
                    nc.scalar.activation(
                        out=q[:], in_=ps[:],
                        func=mybir.ActivationFunctionType.Relu,
                        scale=dh[:, wdx:wdx + 1],
                    )
                    # r = Relu(-dha*ps) = -min(dha*ps, 0); combined below
                    # with subtract so prelu = q - r
                    nc.scalar.activation(
                        out=r[:], in_=ps[:],
                        func=mybir.ActivationFunctionType.Relu,
                        scale=dhan[:, wdx:wdx + 1],
                    )
                    pr = epool.tile([P, H], F32, tag="pr")
                    nc.vector.tensor_tensor(
                        out=pr[:], in0=q[:], in1=r[:],
                        op=mybir.AluOpType.subtract,
                    )
                    return pr
                else:
                    o = epool.tile([P, H], F32, tag="o")
                    nc.vector.tensor_scalar(
                        out=o[:], in0=ps[:], scalar1=dh[:, wdx:wdx + 1],
                        scalar2=None, op0=mybir.AluOpType.mult,
                    )
                    if b_nz:
                        nc.vector.tensor_tensor(
                            out=o[:], in0=o[:], in1=b_t[:, boff:boff + H],
                            op=mybir.AluOpType.add,
                        )
                    nc.vector.tensor_scalar(
                        out=q[:], in0=o[:], scalar1=0.0,
                        scalar2=None, op0=mybir.AluOpType.max,
                    )
                    nc.vector.tensor_scalar(
                        out=r[:], in0=o[:], scalar1=0.0,
                        scalar2=None, op0=mybir.AluOpType.min,
                    )
                    if aun is not None:
                        nc.vector.tensor_scalar_mul(
                            out=r[:], in0=r[:], scalar1=aun)
                    else:
                        nc.vector.tensor_tensor(
                            out=r[:], in0=r[:], in1=a_t[:, boff:boff + H],
                            op=mybir.AluOpType.mult,
                        )
                pr = epool.tile([P, H], F32, tag="pr")
                nc.vector.tensor_tensor(
                    out=pr[:], in0=q[:], in1=r[:], op=mybir.AluOpType.add
                )
                return pr

            def mm(ref, ps, first, last):
                s_tile, g_tile, g = ref
                nc.tensor.matmul(
                    out=ps[:], lhsT=s_tile[:, g * P:(g + 1) * P],
                    rhs=g_tile[:, g * H:(g + 1) * H],
                    start=first, stop=last,
                )

            for w in range(SHARD_TILES):
                # build all selection tiles + issue gathers for this window
                # first, so the PSUM matmul chains then run back-to-back
                refs = []
                for ci in range(2):
                    lo, hi = streams[ci]
                    refs.append((
                        [lo.prep_tile() for _ in range(tpws[ci][0][w])],
                        [hi.prep_tile() for _ in range(tpws[ci][1][w])],
                    ))
                pss = []
                for ci in range(2):
                    ps = wps.tile([P, H], F32, tag=f"p{ci}")
                    lo_refs, hi_refs = refs[ci]
                    for i, r in enumerate(lo_refs):
                        mm(r, ps, i == 0, False)
                    for i, r in enumerate(hi_refs):
                        mm(r, ps, False, i == len(hi_refs) - 1)
                    pss.append(ps)
                p1 = evict(pss[0], dh1, dha1, dhan1, w, a1_uniform, b1_nonzero, 0)
                p2 = evict(pss[1], dh2, dha2, dhan2, w, a2_uniform, b2_nonzero, H)
                ot = epool.tile([P, H], F32, tag="ot")
                nc.vector.tensor_tensor(
                    out=ot[:], in0=p1[:], in1=p2[:], op=mybir.AluOpType.add
                )
                nc.sync.dma_start(out=out[w * P:(w + 1) * P, :], in_=ot[:])

    nc.compile()
    return nc


def kernel(x1, edge_index1, edge_weight1, x2, edge_index2, edge_weight2,
           W1, b1, W2, b2, a1, a2):
    global LAST_EXEC_NS
    g1, meta1 = _prep_graph(x1, edge_index1, edge_weight1, W1, C1)
    g2, meta2 = _prep_graph(x2, edge_index2, edge_weight2, W2, C2)

    b1_nz = bool(np.any(np.asarray(b1) != 0))
    b2_nz = bool(np.any(np.asarray(b2) != 0))
    a1v = np.asarray(a1, np.float32)
    a2v = np.asarray(a2, np.float32)
    a1_uniform = float(a1v.flat[0]) if np.all(a1v == a1v.flat[0]) else None
    a2_uniform = float(a2v.flat[0]) if np.all(a2v == a2v.flat[0]) else None

    nc = _build(meta1, meta2, b1_nz, b2_nz, a1_uniform, a2_uniform)

    iota = np.ascontiguousarray(
        np.broadcast_to(np.arange(P, dtype=np.float32), (P, P))
    ).astype(BF)
    bvec = np.zeros((P, 2 * H), np.float32)
    bvec[:, :H] = np.asarray(b1, np.float32)[None, :]
    bvec[:, H:] = np.asarray(b2, np.float32)[None, :]
    avec = np.zeros((P, 2 * H), np.float32)
    avec[:, :H] = a1v[None, :]
    avec[:, H:] = a2v[None, :]

    a1s = a1_uniform if a1_uniform is not None else 1.0
    a2s = a2_uniform if a2_uniform is not None else 1.0

    in_maps = []
    for k in range(NCORES):
        dh12 = np.zeros((P, 4 * SHARD_TILES), np.float32)
        for ci, g, asc in ((0, g1, a1s), (1, g2, a2s)):
            dv_own = np.ascontiguousarray(
                g["dinv_flat"][k * SHARD:(k + 1) * SHARD]
                .reshape(SHARD_TILES, P).T)
            dh12[:, ci * SHARD_TILES:(ci + 1) * SHARD_TILES] = 0.5 * dv_own
            dh12[:, (2 + ci) * SHARD_TILES:(3 + ci) * SHARD_TILES] = (
                0.5 * asc * dv_own)
        m = {
            "xT1": np.ascontiguousarray(
                g1["xT"][:, k * SHARD:(k + 1) * SHARD]),
            "xT2": np.ascontiguousarray(
                g2["xT"][:, k * SHARD:(k + 1) * SHARD]),
            "W1": g1["W"], "W2": g2["W"],
            "dh12": dh12,
            "iota": iota, "bvec": bvec, "avec": avec,
        }
        for ci, g in ((0, g1), (1, g2)):
            for st in (0, 1):
                m[f"eidx{ci}{st}"] = g["eidx"][st][k]
                m[f"lc{ci}{st}"] = g["lc"][st][k]
                m[f"ew{ci}{st}"] = g["ew"][st][k]
        in_maps.append(m)

    trace = os.environ.get("BASS_KERNEL_TRACE") == "1"
    if trace:
        try:
            import types
            import concourse.bass_utils as bass_utils
            from trn_agent_boot.trn_boot import _ntff_profile_via_ctypes
            _hook = _ntff_profile_via_ctypes("/opt/axon/libaxon_pjrt.so")
            _m = types.ModuleType("antenv.axon_hooks")
            _m.get_axon_ntff_profile_hook = lambda: _hook
            sys.modules["antenv.axon_hooks"] = _m
            bass_utils.upload_artifacts = lambda tmpdir: ""
        except Exception:
            trace = False

    res = run_bass_kernel_spmd(nc, in_maps, core_ids=list(range(NCORES)),
                               trace=trace)
    LAST_EXEC_NS = res.exec_time_ns

    full = np.concatenate([res.results[k]["out"] for k in range(NCORES)],
                          axis=0)
    return np.ascontiguousarray(full[:N])


# revision 13
# speedup vs baseline: 1.0717x; 1.0717x over previous
"""GCN encoder (2x GCNConv + PReLU, averaged) on 8 Trainium2 NeuronCores.

Math (per conv):
    deg[c]  = sum_{e: col_e = c} ew_e + 1
    dinv    = 1/sqrt(deg)
    hhat    = dinv * (x @ W)                       (row-scaled)
    agg[c]  = sum_{e: col_e = c} ew_e * hhat[row_e] + hhat[c]
    out     = prelu(dinv * agg + b, a)
final = (out1 + out2) / 2

v2 vs baseline: the whole aggregation pipeline runs in bf16 (hhat storage,
gathers, and both matmul operands — fp32 matmul costs 4 cycles/row vs 1 for
bf16, and bf16 halves all streaming DMA bytes).  The per-tile selection
matrices S[e, m] = ew_e * (lcol_e == m) are no longer precomputed on the host
and streamed from DRAM (64KB per 128-edge tile); instead each tile's S is
built on-chip by a single fused DVE op
    S = (iota == lcol) * ew        (tensor_scalar, 2 per-partition AP scalars)
from 8 bytes/edge of compact lcol/ew data.  deg -> dinv is computed on the
host (np.bincount) and uploaded (~200KB/graph), dropping the padded degree
tables and the on-device reduction entirely.

Device strategy (unchanged): target nodes sharded over 8 cores (6272 nodes
each on a 128-padded grid, N_PAD = 50176 = 8*49*128). Edges are routed on the
host to the core owning their target col, sorted by col, and laid out as
padded 128-edge tiles grouped into 128-col windows. Self-loops become plain
edges (ew=1, row=col) thanks to the dinv factorization. Source rows are
gathered with the SWDGE dma_gather instruction (int16 indices), so each
conv's edges are split into two streams by source-row half (< / >= N_PAD/2);
both streams of a window accumulate into the same PSUM bank.
"""
import os
import sys

# run_bass_kernel_spmd executes through the axon PJRT platform; if the
# caller pinned jax to cpu, lift that before jax gets imported below.
_jp = os.environ.get("JAX_PLATFORMS")
if _jp is not None and "axon" not in _jp and "neuron" not in _jp:
    del os.environ["JAX_PLATFORMS"]

sys.path.insert(0, "/opt/trn_rl_repo/concourse")
sys.path.insert(0, "/opt/trn_rl_repo")

import ml_dtypes
import numpy as np

import concourse.bass as bass
import concourse.bacc as bacc
import concourse.mybir as mybir
import concourse.tile as tile
from concourse.tile_rust import add_dep_helper
from concourse.bass_utils import run_bass_kernel_spmd

P = 128
N = 50000
NCORES = 8
SHARD_TILES = 49                      # node tiles per core
NT = NCORES * SHARD_TILES             # 392 node tiles
N_PAD = NT * P                        # 50176
HALF = N_PAD // 2                     # 25088 (< 2**15 for int16 gather idx)
SHARD = SHARD_TILES * P               # 6272
C1, C2, H = 512, 256, 256
GD = 16                               # gather blocks (tiles) per dma_gather
NI = P * GD                           # rows per dma_gather
NQ = 4                                # SWDGE queues
CHUNK = 8                             # super-tiles per stream DMA
SPAN = 1024                           # nodes per xT load (2KB/partition rows)
F32 = mybir.dt.float32
BF16 = mybir.dt.bfloat16
I16 = mybir.dt.int16
BF = ml_dtypes.bfloat16

LAST_EXEC_NS = None                   # set when BASS_KERNEL_TRACE=1


def _stream_layout(rows_l, cols, wts, k_arr, w_arr):
    """Slot/pad layout for one (graph, half) edge stream, all cores.

    Returns eidx16 [NCORES,128,NS*NI/16] (wrapped+replicated), lc/ew
    [NCORES,128,NS*GD] f32 (per-slot local col / edge weight, one column
    per 128-edge tile), tpw [SHARD_TILES], NS.
    """
    kw = k_arr * SHARD_TILES + w_arr
    order = np.argsort(kw, kind="stable")
    rows_l, wts, kw = rows_l[order], wts[order], kw[order]
    lcol = (cols[order] & 127).astype(np.float32)

    cw = np.bincount(kw, minlength=NCORES * SHARD_TILES).reshape(
        NCORES, SHARD_TILES
    )
    tpw = np.maximum((cw.max(axis=0) + P - 1) // P, 1).astype(np.int64)
    woff = np.concatenate([[0], np.cumsum(tpw * P)])
    L = int(woff[-1])
    NS = (L // P + GD - 1) // GD
    L_pad = NS * GD * P

    kw_start = np.searchsorted(kw, np.arange(NCORES * SHARD_TILES))
    rank_kw = np.arange(len(kw)) - kw_start[kw]
    slot = woff[kw % SHARD_TILES] + rank_kw

    eidx = np.zeros((NCORES, L_pad), np.int16)
    lc = np.zeros((NCORES, L_pad), np.float32)
    ew = np.zeros((NCORES, L_pad), np.float32)
    k_of = kw // SHARD_TILES
    for k in range(NCORES):
        sel = k_of == k
        sl = slot[sel]
        eidx[k, sl] = rows_l[sel].astype(np.int16)
        lc[k, sl] = lcol[sel]
        ew[k, sl] = wts[sel]

    # per-slot arrays -> [k, p, tile]: slot j = tile*(j//128) lane (j%128)
    ntiles = L_pad // P
    lc_d = np.ascontiguousarray(
        lc.reshape(NCORES, ntiles, P).transpose(0, 2, 1))
    ew_d = np.ascontiguousarray(
        ew.reshape(NCORES, ntiles, P).transpose(0, 2, 1))

    # idx: slot j in super s at (j%16, j//16), replicated over 8 groups
    w16 = eidx.reshape(NCORES, NS, NI // 16, 16).transpose(0, 1, 3, 2)
    w16 = np.tile(w16, (1, 1, 8, 1))                     # [k, NS, 128, NI/16]
    eidx_d = np.ascontiguousarray(w16.transpose(0, 2, 1, 3)).reshape(
        NCORES, P, NS * (NI // 16)
    )
    return eidx_d, lc_d, ew_d, tpw.tolist(), NS


def _prep_graph(x, ei, ew, W, C):
    """Host marshaling for one graph. Returns dict of device arrays + meta."""
    rows0 = np.asarray(ei[0], dtype=np.int64)
    cols0 = np.asarray(ei[1], dtype=np.int64)
    w0 = np.asarray(ew, dtype=np.float32)
    # self loops as plain edges (ew=1, row=col), for every padded node
    loop = np.arange(N_PAD, dtype=np.int64)
    rows = np.concatenate([rows0, loop])
    cols = np.concatenate([cols0, loop])
    wts = np.concatenate([w0, np.ones(N_PAD, np.float32)])

    # deg -> dinv on host (self-loop weights already included)
    deg = np.bincount(cols, weights=wts.astype(np.float64), minlength=N_PAD)
    dinv = (1.0 / np.sqrt(deg)).astype(np.float32)      # deg >= 1 always
    dinv_t = np.ascontiguousarray(dinv.reshape(NT, P).T)  # [P, NT]

    # fold dinv[row] into the edge weight so hhat = x @ W needs no
    # per-node scale on device (self-loop weight 1 becomes dinv[col])
    wts = wts * dinv[rows]

    order = np.argsort(cols, kind="stable")
    rows, cols, wts = rows[order], cols[order], wts[order]

    gt = cols >> 7
    k_arr = gt // SHARD_TILES
    w_arr = gt % SHARD_TILES

    streams = []
    for half in (0, 1):
        sel = (rows >= HALF) == bool(half)
        streams.append(_stream_layout(
            rows[sel] - half * HALF, cols[sel], wts[sel],
            k_arr[sel], w_arr[sel],
        ))

    xT = np.zeros((C, N_PAD), BF)
    xT[:, :N] = np.asarray(x, np.float32).T.astype(BF)
    Wd = np.ascontiguousarray(
        np.asarray(W, np.float32).astype(BF).reshape(
            C // P, P, H).transpose(1, 0, 2)
    ).reshape(P, (C // P) * H)

    meta = {
        "tpw": [streams[0][3], streams[1][3]],
        "NS": [streams[0][4], streams[1][4]],
    }
    arrs = {
        "dinv": dinv_t, "dinv_flat": dinv,
        "eidx": [streams[0][0], streams[1][0]],
        "lc": [streams[0][1], streams[1][1]],
        "ew": [streams[0][2], streams[1][2]],
        "xT": xT, "W": Wd,
    }
    return arrs, meta


def _build(meta1, meta2, b1_nonzero, b2_nonzero, a1_uniform, a2_uniform):
    nc = bacc.Bacc("TRN2", target_bir_lowering=False, debug=False,
                   num_devices=NCORES, num_swdge_queues=NQ)
    Pool = mybir.EngineType.Pool

    # ---- DRAM parameters ----
    xT1 = nc.declare_dram_parameter("xT1", [C1, SHARD], BF16, isOutput=False)
    xT2 = nc.declare_dram_parameter("xT2", [C2, SHARD], BF16, isOutput=False)
    W1 = nc.declare_dram_parameter("W1", [P, (C1 // P) * H], BF16, isOutput=False)
    W2 = nc.declare_dram_parameter("W2", [P, (C2 // P) * H], BF16, isOutput=False)
    dh12 = nc.declare_dram_parameter("dh12", [P, 4 * SHARD_TILES], F32,
                                     isOutput=False)
    eidx, lcp, ewp = [], [], []
    for ci, meta in ((0, meta1), (1, meta2)):
        for st in (0, 1):
            NS = meta["NS"][st]
            eidx.append(nc.declare_dram_parameter(
                f"eidx{ci}{st}", [P, NS * (NI // 16)], I16, isOutput=False))
            lcp.append(nc.declare_dram_parameter(
                f"lc{ci}{st}", [P, NS * GD], F32, isOutput=False))
            ewp.append(nc.declare_dram_parameter(
                f"ew{ci}{st}", [P, NS * GD], F32, isOutput=False))
    iota_in = nc.declare_dram_parameter("iota", [P, P], BF16, isOutput=False)
    bvec = nc.declare_dram_parameter("bvec", [P, 2 * H], F32, isOutput=False)
    avec = nc.declare_dram_parameter("avec", [P, 2 * H], F32, isOutput=False)
    out = nc.declare_dram_parameter("out", [SHARD, H], F32, isOutput=True)

    hh1o = nc.dram_tensor("hh1o", [SHARD, H], BF16)
    hh2o = nc.dram_tensor("hh2o", [SHARD, H], BF16)
    hh1 = nc.dram_tensor("hh1", [N_PAD, H], BF16)
    hh2 = nc.dram_tensor("hh2", [N_PAD, H], BF16)

    qrr = [0]  # round-robin SWDGE queue counter

    with tile.TileContext(nc) as tc:
        with (
            tc.tile_pool(name="const", bufs=1) as cpool,
            tc.tile_pool(name="dinv", bufs=1) as dvpool,
            tc.tile_pool(name="xin", bufs=3) as xpool,
            tc.tile_pool(name="hout", bufs=2) as hpool,
            tc.tile_pool(name="stream", bufs=2) as spool,
            tc.tile_pool(name="gath", bufs=2) as gpool,
            tc.tile_pool(name="sld", bufs=3) as sldpool,
            tc.tile_pool(name="evict", bufs=2) as epool,
            tc.tile_pool(name="hps", bufs=2, space="PSUM") as hps,
            tc.tile_pool(name="wps", bufs=2, space="PSUM") as wps,
        ):
            iota_t = cpool.tile([P, P], BF16)
            nc.sync.dma_start(out=iota_t[:], in_=iota_in[:])
            w1_t = cpool.tile([P, (C1 // P) * H], BF16)
            nc.sync.dma_start(out=w1_t[:], in_=W1[:])
            w2_t = cpool.tile([P, (C2 // P) * H], BF16)
            nc.sync.dma_start(out=w2_t[:], in_=W2[:])
            b_t = cpool.tile([P, 2 * H], F32)
            if b1_nonzero or b2_nonzero:
                nc.sync.dma_start(out=b_t[:], in_=bvec[:])
            a_t = cpool.tile([P, 2 * H], F32)
            if a1_uniform is None or a2_uniform is None:
                nc.sync.dma_start(out=a_t[:], in_=avec[:])

            # ---- dinv-derived evict scales (host-computed) ----
            dh_t = dvpool.tile([P, 4 * SHARD_TILES], F32)
            nc.sync.dma_start(out=dh_t[:], in_=dh12[:])
            ST = SHARD_TILES
            dh1 = dh_t[:, 0:ST]
            dh2 = dh_t[:, ST:2 * ST]
            dha1 = dh_t[:, 2 * ST:3 * ST]
            dha2 = dh_t[:, 3 * ST:4 * ST]
            dhan_t = dvpool.tile([P, 2 * ST], F32)
            nc.vector.tensor_scalar_mul(
                out=dhan_t[:], in0=dh_t[:, 2 * ST:4 * ST], scalar1=-1.0)
            dhan1 = dhan_t[:, 0:ST]
            dhan2 = dhan_t[:, ST:2 * ST]

            # ---- hhat = x @ W (dinv folded into edge weights) ----
            PSPAN = 4                 # node tiles per PSUM tile (2 banks)

            def h_phase(xT, w_t, nck, hh, lo, hi):
                writes = []
                for n0 in range(lo, hi, SPAN):
                    n1 = min(n0 + SPAN, hi)
                    xt = xpool.tile([P, nck * (n1 - n0)], BF16, tag="xt")
                    nc.sync.dma_start(
                        out=xt[:].rearrange("p (c n) -> p c n", c=nck),
                        in_=xT[:, n0:n1].rearrange("(c p) n -> p c n", p=P),
                    )
                    njt = (n1 - n0) // P
                    for j0 in range(0, njt, PSPAN):
                        jn = min(PSPAN, njt - j0)
                        ps = hps.tile([P, PSPAN * H], F32, tag="hps")
                        for j in range(j0, j0 + jn):
                            for c in range(nck):
                                nc.tensor.matmul(
                                    out=ps[:, (j - j0) * H:(j - j0 + 1) * H],
                                    lhsT=xt[:, c * (n1 - n0) + j * P:
                                            c * (n1 - n0) + (j + 1) * P],
                                    rhs=w_t[:, c * H:(c + 1) * H],
                                    start=(c == 0), stop=(c == nck - 1),
                                )
                        ht = hpool.tile([P, PSPAN * H], BF16, tag="ht")
                        nc.scalar.activation(
                            out=ht[:, :jn * H], in_=ps[:, :jn * H],
                            func=mybir.ActivationFunctionType.Copy,
                        )
                        t0 = (n0 // P) + j0
                        wr = nc.sync.dma_start(
                            out=hh[t0 * P:(t0 + jn) * P, :].rearrange(
                                "(t p) h -> p t h", p=P),
                            in_=ht[:, :jn * H].rearrange(
                                "p (t h) -> p t h", h=H),
                        )
                        writes.append(wr)
                return writes

            # compute only this core's node shard, then AllGather the full
            # hhat across the 8 cores (replicated result in each local hh)
            wr1 = h_phase(xT1, w1_t, C1 // P, hh1o, 0, SHARD)
            wr2 = h_phase(xT2, w2_t, C2 // P, hh2o, 0, SHARD)

            barriers = {}
            rg = [list(range(NCORES))]
            for key, wrs, own, full in [("1", wr1, hh1o, hh1),
                                        ("2", wr2, hh2o, hh2)]:
                cc = nc.gpsimd.collective_compute(
                    kind="AllGather",
                    op=mybir.AluOpType.bypass,
                    replica_groups=rg,
                    ins=[own[:, :]],
                    outs=[full[:, :]],
                )
                for w in wrs:
                    add_dep_helper(cc.ins, w.ins, reason=f"hh{key} own write")
                bar = nc.engines[Pool].nop(nofuse=True, hint=f"hh{key}_ready")
                add_dep_helper(bar.ins, cc.ins, reason=f"hh{key} allgather")
                barriers[key] = bar

            # ---- edge streams ----
            class Stream:
                def __init__(self, eidx, lcp, ewp, hh_half, NS, barrier, tag):
                    self.eidx, self.lcp, self.ewp = eidx, lcp, ewp
                    self.hh = hh_half
                    self.NS, self.barrier, self.tag = NS, barrier, tag
                    self.t_mm = 0
                    self.chunk_base = 0
                    self.idx_tile = None
                    self.lc_tile = None
                    self.ew_tile = None
                    self.s_tile = None
                    self.g_tile = None

                def ensure(self):
                    s, g = divmod(self.t_mm, GD)
                    if s % CHUNK == 0 and g == 0:
                        s1 = min(s + CHUNK, self.NS)
                        it = spool.tile([P, (s1 - s) * (NI // 16)], I16,
                                        tag=f"idx{self.tag}")
                        nc.sync.dma_start(
                            out=it[:],
                            in_=self.eidx[:, s * (NI // 16):s1 * (NI // 16)])
                        self.idx_tile = it
                        lt = spool.tile([P, (s1 - s) * GD], F32,
                                        tag=f"lc{self.tag}")
                        nc.sync.dma_start(
                            out=lt[:], in_=self.lcp[:, s * GD:s1 * GD])
                        self.lc_tile = lt
                        et = spool.tile([P, (s1 - s) * GD], F32,
                                        tag=f"ew{self.tag}")
                        nc.sync.dma_start(
                            out=et[:], in_=self.ewp[:, s * GD:s1 * GD])
                        self.ew_tile = et
                        self.chunk_base = s
                    if g == 0:
                        so = s - self.chunk_base
                        gt_ = gpool.tile([P, GD * H], BF16, tag=f"g{self.tag}")
                        gi = nc.gpsimd.dma_gather(
                            out_ap=gt_[:].rearrange("p (b e) -> p b e", e=H),
                            in_ap=self.hh,
                            idxs_ap=self.idx_tile[
                                :, so * (NI // 16):(so + 1) * (NI // 16)],
                            num_idxs=NI,
                            num_idxs_reg=NI,
                            elem_size=H,
                            queue_num=qrr[0] % NQ,
                            single_packet=False,
                        )
                        qrr[0] += 1
                        add_dep_helper(
                            gi.ins, self.barrier.ins, reason="hh ready")
                        self.g_tile = gt_
                        st_ = sldpool.tile([P, GD * P], BF16,
                                           tag=f"sl{self.tag}")
                        self.s_tile = st_

                def prep_tile(self):
                    self.ensure()
                    s, g = divmod(self.t_mm, GD)
                    col = (s - self.chunk_base) * GD + g
                    # S[e, m] = ew_e * (lcol_e == m), built on-chip
                    nc.vector.tensor_scalar(
                        out=self.s_tile[:, g * P:(g + 1) * P],
                        in0=iota_t[:],
                        scalar1=self.lc_tile[:, col:col + 1],
                        scalar2=self.ew_tile[:, col:col + 1],
                        op0=mybir.AluOpType.is_equal,
                        op1=mybir.AluOpType.mult,
                    )
                    self.t_mm += 1
                    return (self.s_tile, self.g_tile, g)

            streams = []
            for ci, (meta, hh) in enumerate([(meta1, hh1), (meta2, hh2)]):
                pair = []
                for st in (0, 1):
                    half_ap = hh[st * HALF:(st + 1) * HALF, :]
                    bar = barriers[f"{ci + 1}"]
                    pair.append(Stream(
                        eidx[ci * 2 + st], lcp[ci * 2 + st], ewp[ci * 2 + st],
                        half_ap, meta["NS"][st], bar, f"{ci}{st}"))
                streams.append(pair)

            # per-stream tiles per window, with drain pads in the last window
            tpws = []
            for ci, meta in ((0, meta1), (1, meta2)):
                pair = []
                for st in (0, 1):
                    tp = list(meta["tpw"][st])
                    tp[-1] += meta["NS"][st] * GD - sum(tp)
                    pair.append(tp)
                tpws.append(pair)

            def evict(ps, dh, dha, dhan, wdx, aun, b_nz, boff):
                q = epool.tile([P, H], F32, tag="q")
                r = epool.tile([P, H], F32, tag="r")
                if not b_nz and aun is not None:
                    nc.scalar.activation(
                        out=q[:], in_=ps[:],
                        func=mybir.ActivationFunctionType.Relu,
                        scale=dh[:, wdx:wdx + 1],
                    )
                    # r = Relu(-dha*ps) = -min(dha*ps, 0); combined below
                    # with subtract so prelu = q - r
                    nc.scalar.activation(
                        out=r[:], in_=ps[:],
                        func=mybir.ActivationFunctionType.Relu,
                        scale=dhan[:, wdx:wdx + 1],
                    )
                    pr = epool.tile([P, H], F32, tag="pr")
                    nc.vector.tensor_tensor(
                        out=pr[:], in0=q[:], in1=r[:],
                        op=mybir.AluOpType.subtract,
                    )
                    return pr
                else:
                    o = epool.tile([P, H], F32, tag="o")
                    nc.vector.tensor_scalar(
                        out=o[:], in0=ps[:], scalar1=dh[:, wdx:wdx + 1],
                        scalar2=None, op0=mybir.AluOpType.mult,
                    )
                    if b_nz:
                        nc.vector.tensor_tensor(
                            out=o[:], in0=o[:], in1=b_t[:, boff:boff + H],
                            op=mybir.AluOpType.add,
                        )
                    nc.vector.tensor_scalar(
                        out=q[:], in0=o[:], scalar1=0.0,
                        scalar2=None, op0=mybir.AluOpType.max,
                    )
                    nc.vector.tensor_scalar(
                        out=r[:], in0=o[:], scalar1=0.0,
                        scalar2=None, op0=mybir.AluOpType.min,
                    )
                    if aun is not None:
                        nc.vector.tensor_scalar_mul(
                            out=r[:], in0=r[:], scalar1=aun)
                    else:
                        nc.vector.tensor_tensor(
                            out=r[:], in0=r[:], in1=a_t[:, boff:boff + H],
                            op=mybir.AluOpType.mult,
                        )
                pr = epool.tile([P, H], F32, tag="pr")
                nc.vector.tensor_tensor(
                    out=pr[:], in0=q[:], in1=r[:], op=mybir.AluOpType.add
                )
                return pr

            def mm(ref, ps, first, last):
                s_tile, g_tile, g = ref
                nc.tensor.matmul(
                    out=ps[:], lhsT=s_tile[:, g * P:(g + 1) * P],
                    rhs=g_tile[:, g * H:(g + 1) * H],
                    start=first, stop=last,
                )

            for w in range(SHARD_TILES):
                # build all selection tiles + issue gathers for this window
                # first, so the PSUM matmul chains then run back-to-back
                refs = []
                for ci in range(2):
                    lo, hi = streams[ci]
                    refs.append((
                        [lo.prep_tile() for _ in range(tpws[ci][0][w])],
                        [hi.prep_tile() for _ in range(tpws[ci][1][w])],
                    ))
                pss = []
                for ci in range(2):
                    ps = wps.tile([P, H], F32, tag=f"p{ci}")
                    lo_refs, hi_refs = refs[ci]
                    for i, r in enumerate(lo_refs):
                        mm(r, ps, i == 0, False)
                    for i, r in enumerate(hi_refs):
                        mm(r, ps, False, i == len(hi_refs) - 1)
                    pss.append(ps)
                p1 = evict(pss[0], dh1, dha1, dhan1, w, a1_uniform, b1_nonzero, 0)
                p2 = evict(pss[1], dh2, dha2, dhan2, w, a2_uniform, b2_nonzero, H)
                ot = epool.tile([P, H], F32, tag="ot")
                nc.vector.tensor_tensor(
                    out=ot[:], in0=p1[:], in1=p2[:], op=mybir.AluOpType.add
                )
                nc.sync.dma_start(out=out[w * P:(w + 1) * P, :], in_=ot[:])

    nc.compile()
    return nc


def kernel(x1, edge_index1, edge_weight1, x2, edge_index2, edge_weight2,
           W1, b1, W2, b2, a1, a2):
    global LAST_EXEC_NS
    g1, meta1 = _prep_graph(x1, edge_index1, edge_weight1, W1, C1)
    g2, meta2 = _prep_graph(x2, edge_index2, edge_weight2, W2, C2)

    b1_nz = bool(np.any(np.asarray(b1) != 0))
    b2_nz = bool(np.any(np.asarray(b2) != 0))
    a1v = np.asarray(a1, np.float32)
    a2v = np.asarray(a2, np.float32)
    a1_uniform = float(a1v.flat[0]) if np.all(a1v == a1v.flat[0]) else None
    a2_uniform = float(a2v.flat[0]) if np.all(a2v == a2v.flat[0]) else None

    nc = _build(meta1, meta2, b1_nz, b2_nz, a1_uniform, a2_uniform)

    iota = np.ascontiguousarray(
        np.broadcast_to(np.arange(P, dtype=np.float32), (P, P))
    ).astype(BF)
    bvec = np.zeros((P, 2 * H), np.float32)
    bvec[:, :H] = np.asarray(b1, np.float32)[None, :]
    bvec[:, H:] = np.asarray(b2, np.float32)[None, :]
    avec = np.zeros((P, 2 * H), np.float32)
    avec[:, :H] = a1v[None, :]
    avec[:, H:] = a2v[None, :]

    a1s = a1_uniform if a1_uniform is not None else 1.0
    a2s = a2_uniform if a2_uniform is not None else 1.0

    in_maps = []
    for k in range(NCORES):
        dh12 = np.zeros((P, 4 * SHARD_TILES), np.float32)
        for ci, g, asc in ((0, g1, a1s), (1, g2, a2s)):
            dv_own = np.ascontiguousarray(
                g["dinv_flat"][k * SHARD:(k + 1) * SHARD]
                .reshape(SHARD_TILES, P).T)
            dh12[:, ci * SHARD_TILES:(ci + 1) * SHARD_TILES] = 0.5 * dv_own
            dh12[:, (2 + ci) * SHARD_TILES:(3 + ci) * SHARD_TILES] = (
                0.5 * asc * dv_own)
        m = {
            "xT1": np.ascontiguousarray(
                g1["xT"][:, k * SHARD:(k + 1) * SHARD]),
            "xT2": np.ascontiguousarray(
                g2["xT"][:, k * SHARD:(k + 1) * SHARD]),
            "W1": g1["W"], "W2": g2["W"],
            "dh12": dh12,
            "iota": iota, "bvec": bvec, "avec": avec,
        }
        for ci, g in ((0, g1), (1, g2)):
            for st in (0, 1):
                m[f"eidx{ci}{st}"] = g["eidx"][st][k]
                m[f"lc{ci}{st}"] = g["lc"][st][k]
                m[f"ew{ci}{st}"] = g["ew"][st][k]
        in_maps.append(m)

    trace = os.environ.get("BASS_KERNEL_TRACE") == "1"
    if trace:
        try:
            import types
            import concourse.bass_utils as bass_utils
            from trn_agent_boot.trn_boot import _ntff_profile_via_ctypes
            _hook = _ntff_profile_via_ctypes("/opt/axon/libaxon_pjrt.so")
            _m = types.ModuleType("antenv.axon_hooks")
            _m.get_axon_ntff_profile_hook = lambda: _hook
            sys.modules["antenv.axon_hooks"] = _m
            bass_utils.upload_artifacts = lambda tmpdir: ""
        except Exception:
            trace = False

    res = run_bass_kernel_spmd(nc, in_maps, core_ids=list(range(NCORES)),
                               trace=trace)
    LAST_EXEC_NS = res.exec_time_ns

    full = np.concatenate([res.results[k]["out"] for k in range(NCORES)],
                          axis=0)
    return np.ascontiguousarray(full[:N])
